# revision 1
# baseline (speedup 1.0000x reference)
"""CondConv2D Trainium2 kernel (v2).

Problem (hardcoded): B=16, C_in=64, H=W=256, E=4, C_out=64, 3x3, s=1, d=1, p=1.
Sharding: data-parallel over batch, 8 cores x 2 images.

v2 changes vs v1:
  - fp32 HWDGE loads into a small staging pool; ACT does a fused
    fp32->bf16 cast + per-tile pooling reduction (activation accum_out),
    writing padded persistent bf16 tiles [128, 13, 258] (zero pad cols).
    No SWDGE cast DMAs, no DVE reduces.
  - All conv matmuls are full N=512 (edge columns come from the zero pad
    cols), removing the 255-col split matmuls that dominated v1.
  - 20 persistent image tiles (2 images x 10) - image i+1 loads/casts
    overlap image i's conv with no slot-rotation deadlocks.
  - Stores issued on the gpsimd (SWDGE) queue so they never queue behind
    loads (sync) or casts (scalar).
"""
import sys

if "/opt/trn_rl_repo" not in sys.path:
    sys.path.insert(0, "/opt/trn_rl_repo")

import numpy as np

import concourse.bacc as bacc
import concourse.mybir as mybir
import concourse.tile as tile
from concourse.bass_utils import run_bass_kernel_spmd

F32 = mybir.dt.float32
BF16 = mybir.dt.bfloat16
AF = mybir.ActivationFunctionType
ALU = mybir.AluOpType

N_CORES = 8
IMGS = 2
C_IN = 64
C_OUT = 64
H = 256
W = 256
E = 4
NTAP = 9
RPT = 13           # rows per tile
NT = 10            # tiles per image (130 rows per half: -1..128 / 127..256)
HALF = 128
STAGE_ROWS = 16


def build_nc():
    nc = bacc.Bacc("TRN2", target_bir_lowering=False, debug=False,
                   num_devices=N_CORES)
    # xp: host-prepared tile layout. Partition p<64: top-half channels,
    # row r = x row r-1 (row 0 = zero pad); p>=64: bottom-half channels,
    # row r = x row 127+r (row 129 = zero pad).
    x = nc.dram_tensor("xp", [IMGS, 128, 130, W], F32, kind="ExternalInput")
    wt = nc.dram_tensor("wt", [128, E * NTAP * C_OUT], F32,
                        kind="ExternalInput")
    fcw = nc.dram_tensor("fcw", [128, E], F32, kind="ExternalInput")
    fcb = nc.dram_tensor("fcb", [128, E], F32, kind="ExternalInput")
    ones = nc.dram_tensor("ones", [128, 128], F32, kind="ExternalInput")
    y = nc.dram_tensor("y", [IMGS, C_OUT, H, W], F32, kind="ExternalOutput")

    S = NTAP * C_OUT  # 576

    with tile.TileContext(nc) as tc:
        with (
            tc.tile_pool(name="consts", bufs=1) as consts,
            tc.tile_pool(name="stgp", bufs=3) as stgp,
            tc.tile_pool(name="small", bufs=2) as small,
            tc.tile_pool(name="stage", bufs=2) as stage_pool,
            tc.tile_pool(name="psum", bufs=1, space="PSUM") as psum_pool,
        ):
            # ---- consts ----
            wtmp = stgp.tile([128, E * S], F32, tag="stg",
                             padded_shape=[128, RPT * W])
            nc.sync.dma_start(wtmp[:], wt[:])
            wtb = consts.tile([128, E * S], BF16)
            nc.scalar.activation(wtb[:], wtmp[:], AF.Copy)
            fcwt = consts.tile([128, E], F32)
            fcbt = consts.tile([128, E], F32)
            onest = consts.tile([128, 128], F32)
            nc.sync.dma_start(fcwt[:], fcw[:])
            nc.sync.dma_start(fcbt[:], fcb[:])
            nc.sync.dma_start(onest[:], ones[:])

            # ---- persistent image tiles; memset pads once ----
            xs = [[consts.tile([128, RPT, 258], BF16, name=f"xs{i}_{t}")
                   for t in range(NT)] for i in range(IMGS)]
            for i in range(IMGS):
                for t in range(NT):
                    nc.vector.memset(xs[i][t][:, :, 0:1], 0.0)
                    nc.vector.memset(xs[i][t][:, :, 257:258], 0.0)
                # top half: row -1 pad; bottom half: row 256 pad
                nc.vector.memset(xs[i][0][0:64, 0:1, :], 0.0)
                nc.vector.memset(xs[i][NT - 1][64:128, 12:13, :], 0.0)

            # per-image routing partials (13 cast ops -> 13 cols used)
            partials = [small.tile([128, 16], F32, name=f"par{i}", tag="par",
                                   bufs=2) for i in range(IMGS)]
            for i in range(IMGS):
                nc.vector.memset(partials[i][:], 0.0)
            # zeros tile so DVE drains can use tensor_tensor (which never
            # enters the 2-port perf mode that starves SWDGE stores)
            zdrain = consts.tile([128, 2, W], F32, name="zdrain")
            nc.vector.memset(zdrain[:], 0.0)

            def emit_tile(i, t, col, dve_tiles, paced=False):
                par = partials[i]

                def cast(dst_rows, src, hs, acc=True):
                    t_, r0, r1 = dst_rows
                    kw = {}
                    if acc:
                        kw["accum_out"] = par[hs, col[0]:col[0] + 1]
                        col[0] += 1
                    nc.scalar.activation(
                        xs[i][t_][hs, r0:r1, 1:257], src, AF.Copy, **kw)

                stg = stgp.tile([128, RPT, W], F32, tag="stg")
                if paced:
                    # marker write: the load DMA (WAW on this tile) cannot
                    # start until this DVE op runs, which sits in the DVE
                    # queue behind the preceding conv drains - pacing image
                    # 1's loads to conv0 progress so stores never starve.
                    nc.vector.memset(stg[0:1, 0:1, 0:1], 0.0)
                nc.sync.dma_start(stg[:], x[i, :, 13 * t:13 * t + 13, :])
                if t == 0:
                    cast((0, 0, 13), stg[0:64], slice(0, 64))
                    # bottom rows 0,1 are x rows 127,128, already counted
                    # by the top half - exclude from pooling accumulators.
                    cast((0, 0, 2), stg[64:128, 0:2, :], slice(64, 128),
                         acc=False)
                    cast((0, 2, 13), stg[64:128, 2:13, :], slice(64, 128))
                elif t in dve_tiles:
                    nc.vector.tensor_copy(xs[i][t][:, 0:13, 1:257], stg[:])
                    nc.vector.reduce_sum(par[:, col[0]:col[0] + 1],
                                         xs[i][t][:, 0:13, :],
                                         axis=mybir.AxisListType.XY)
                    col[0] += 1
                else:
                    cast((t, 0, 13), stg[:], slice(0, 128))

            def load_image(i):
                # image 0 loads have nothing to overlap with, so split its
                # casts across ACT and DVE (DVE is otherwise idle there).
                # Image 1's casts stay on ACT: they overlap conv0, whose
                # drains live on DVE.
                dve_tiles = {1, 3, 5, 7} if i == 0 else set()
                col = [0]
                for t in range(NT):
                    emit_tile(i, t, col, dve_tiles)

            def routing(i):
                par = partials[i]
                pooled = small.tile([128, 1], F32, name="pooled")
                nc.vector.reduce_sum(pooled[:], par[:],
                                     axis=mybir.AxisListType.X)
                tmp4 = small.tile([128, E], F32, name="tmp4")
                nc.vector.tensor_scalar(tmp4[:], fcwt[:], pooled[:, 0:1],
                                        1.0 / float(H * W),
                                        op0=ALU.mult, op1=ALU.mult)
                ps4 = psum_pool.tile([128, E], F32, name="ps4", tag="rt",
                                     bufs=1)
                nc.tensor.matmul(ps4[0:64], onest[0:64, 0:64], tmp4[0:64],
                                 start=True, stop=True, tile_position=(0, 0),
                                 skip_group_check=True)
                nc.tensor.matmul(ps4[64:128], onest[64:128, 64:128],
                                 tmp4[64:128], start=True, stop=True,
                                 tile_position=(64, 64), skip_group_check=True)
                logits = small.tile([128, E], F32, name="logits")
                nc.vector.tensor_tensor(logits[:], ps4[:], fcbt[:], op=ALU.add)
                rt = small.tile([128, E], F32, name="rt")
                nc.scalar.activation(rt[:], logits[:], AF.Sigmoid)
                wmix = small.tile([128, S], BF16, name="wmix", tag="wmix")
                nc.vector.tensor_scalar_mul(wmix[:], wtb[:, 0:S], rt[:, 0:1])
                for e in range(1, E):
                    nc.vector.scalar_tensor_tensor(
                        wmix[:], wtb[:, e * S:(e + 1) * S], rt[:, e:e + 1],
                        wmix[:], op0=ALU.mult, op1=ALU.add)
                return wmix

            def conv(i, wmix, on_fill=None):
                xi = xs[i]
                n_groups = 32           # 2 pairs per group
                gps = STAGE_ROWS // 4   # groups per stage tile (4)
                stage = None
                for g in range(n_groups):
                    if g % gps == 0:
                        # bf16 stage: the SWDGE store DMAs cast bf16->fp32 on
                        # the way to HBM (write traffic unchanged), halving
                        # the stage SBUF footprint to pay for stg bufs=3.
                        stage = stage_pool.tile([128, STAGE_ROWS, W], BF16,
                                                name="stage", tag="st")
                    psA = psum_pool.tile([128, 2, W], F32, name="psA",
                                         tag="ps", bufs=6)
                    psB = psum_pool.tile([128, 2, W], F32, name="psB",
                                         tag="ps", bufs=6)
                    pstiles = (psA, psB)
                    # last tap must be unsplit for both pairs: pick clean kh
                    bad = set()
                    for px in range(2):
                        pair = 2 * g + px
                        for kh in range(3):
                            if (2 * pair + kh) % RPT == RPT - 1:
                                bad.add(kh)
                    clean = [kh for kh in range(3) if kh not in bad][-1]
                    khs = [kh for kh in range(3) if kh != clean] + [clean]
                    taps = [kh * 3 + kw for kh in khs for kw in range(3)]
                    for r, tap in enumerate(taps):
                        kh, kw = divmod(tap, 3)
                        st = r == 0
                        sp = r == len(taps) - 1
                        for px in range(2):
                            pair = 2 * g + px
                            L = 2 * pair + kh
                            t, m = divmod(L, RPT)
                            ps = pstiles[px]
                            for half in range(2):
                                hs = slice(0, 64) if half == 0 else \
                                    slice(64, 128)
                                lhsT = wmix[hs, tap * 64:(tap + 1) * 64]
                                if px == 0:
                                    tp = (0, 0) if half == 0 else (64, 64)
                                    osl = hs
                                else:
                                    tp = (0, 64) if half == 0 else (64, 0)
                                    osl = slice(64, 128) if half == 0 else \
                                        slice(0, 64)
                                if m <= RPT - 2:
                                    rhs = xi[t][hs, m:m + 2, kw:kw + 256]
                                    nc.tensor.matmul(
                                        ps[osl], lhsT, rhs, start=st, stop=sp,
                                        tile_position=tp,
                                        skip_group_check=True)
                                else:
                                    for j in range(2):
                                        tj, mj = divmod(L + j, RPT)
                                        rhs = xi[tj][hs, mj, kw:kw + 256]
                                        nc.tensor.matmul(
                                            ps[osl, j, :], lhsT, rhs,
                                            start=(st and j == 0), stop=sp,
                                            tile_position=tp,
                                            skip_group_check=True)
                    # drain psum -> staging (alternate DVE / ACT)
                    r0 = (g % gps) * 4
                    for px in range(2):
                        dst = stage[:, r0 + 2 * px:r0 + 2 * px + 2, :]
                        src = pstiles[px][:]
                        # all drains on DVE: the ACT queue is busy with the
                        # next image's casts during conv, and tensor_tensor
                        # never enters the 2-port mode that starves SWDGE.
                        nc.vector.tensor_tensor(dst, src, zdrain[:],
                                                op=ALU.add)
                    # stage full -> 4 interleaved store DMAs on gpsimd
                    if (g + 1) % gps == 0:
                        mrow = (g // gps) * STAGE_ROWS
                        nj = STAGE_ROWS // 4
                        sv = stage.rearrange("p (j b r) w -> p j b r w",
                                             j=nj, b=2, r=2)
                        ys = y[i].rearrange("c (blk four) w -> c blk four w",
                                            four=4)
                        nc.gpsimd.dma_start(
                            ys[:, mrow // 4:mrow // 4 + nj, 0:2, :],
                            sv[0:64, :, 0, :, :])
                        nc.gpsimd.dma_start(
                            ys[:, (HALF + mrow) // 4:(HALF + mrow) // 4 + nj,
                               2:4, :],
                            sv[0:64, :, 1, :, :])
                        nc.gpsimd.dma_start(
                            ys[:, (HALF + mrow) // 4:(HALF + mrow) // 4 + nj,
                               0:2, :],
                            sv[64:128, :, 0, :, :])
                        nc.gpsimd.dma_start(
                            ys[:, mrow // 4:mrow // 4 + nj, 2:4, :],
                            sv[64:128, :, 1, :, :])
                        if on_fill is not None:
                            on_fill(g // gps)

            # ---- schedule ----
            # routing(0) is emitted before load_image(1) so image 0's
            # routing ops are not stuck behind image 1's casts in the
            # ACT/DVE FIFO queues. Image 1's tiles 2..9 are emitted at
            # conv0's fill boundaries with a pacing marker so their load
            # DMAs trail conv0 progress instead of monopolizing the SDMA
            # engines ahead of conv0's stores.
            load_image(0)
            wmix0 = routing(0)
            col1 = [0]
            emit_tile(1, 0, col1, set())
            emit_tile(1, 1, col1, set())

            def on_fill(f):
                t = f + 2
                if t < NT:
                    emit_tile(1, t, col1, set(), paced=True)

            conv(0, wmix0, on_fill=on_fill)
            wmix1 = routing(1)
            conv(1, wmix1)

    nc.compile()
    return nc


_NC_CACHE = {}


def _get_nc():
    if "nc" not in _NC_CACHE:
        _NC_CACHE["nc"] = build_nc()
    return _NC_CACHE["nc"]


def _prep_x(x2):
    """[2, 64, 256, 256] -> tile layout [2, 128, 130, 256] (pads baked)."""
    xp = np.zeros((IMGS, 128, 130, W), dtype=np.float32)
    xp[:, 0:64, 1:130, :] = x2[:, :, 0:129, :]
    xp[:, 64:128, 0:129, :] = x2[:, :, 127:256, :]
    return xp


def _prep_shared(weight, fc_w, fc_b):
    # [E, O, I, KH, KW] -> [I, E, KH, KW, O] -> [64, E*9*64], dup halves
    wt = np.ascontiguousarray(weight.transpose(2, 0, 3, 4, 1)).reshape(
        C_IN, E * NTAP * C_OUT)
    wt = np.concatenate([wt, wt], axis=0).astype(np.float32)
    fcw = np.concatenate([fc_w.T, fc_w.T], axis=0).astype(np.float32)
    fcb = np.tile(fc_b.reshape(1, E), (128, 1)).astype(np.float32)
    ones = np.ones((128, 128), np.float32)
    return wt, fcw, fcb, ones


def kernel(inputs, weight, fc_w, fc_b, stride=1, dilation=1, padding=1,
           _trace=False, _npx=2):
    assert int(stride) == 1 and int(dilation) == 1 and int(padding) == 1
    inputs = np.asarray(inputs, dtype=np.float32)
    B = inputs.shape[0]
    assert B == N_CORES * IMGS
    wt, fcw, fcb, ones = _prep_shared(np.asarray(weight), np.asarray(fc_w),
                                      np.asarray(fc_b))
    nc = _get_nc()
    in_maps = []
    for c in range(N_CORES):
        in_maps.append({
            "xp": _prep_x(inputs[2 * c:2 * c + 2]),
            "wt": wt, "fcw": fcw, "fcb": fcb, "ones": ones,
        })
    res = run_bass_kernel_spmd(nc, in_maps, core_ids=list(range(N_CORES)),
                               trace=_trace)
    out = np.concatenate([res.results[c]["y"] for c in range(N_CORES)], axis=0)
    if _trace:
        return out, res
    return out



# revision 8
# speedup vs baseline: 1.0322x; 1.0322x over previous
"""CondConv2D Trainium2 kernel (v3).

Problem (hardcoded): B=16, C_in=64, H=W=256, E=4, C_out=64, 3x3, s=1, d=1, p=1.
Sharding: data-parallel over batch, 8 cores x 2 images.

v3 changes vs v2:
  - bf16 HBM I/O. Input is host-cast to bf16 in the padded tile layout
    (row AND col pads baked host-side), so loads are pure 128x6.7KB
    block copies: no device casts, no pad memsets, half the read
    traffic. Output dram tensor is bf16 (host upcasts to fp32): half
    the write traffic. Total HBM traffic 69MB -> 35MB.
  - Pooling is done by dedicated reduce ops split across DVE
    (tensor_reduce) and ACT (activation-Copy with accum_out into a
    scratch tile), interleaved with the loads / previous image's conv.
  - Routing fixed to sum top+bottom halves (4 ones-matmuls with
    cross tile_positions) instead of per-half logits.
  - Image 1 loads eagerly behind image 0 (bf16 halves the bandwidth
    demand, so loads+stores fit under conv0), its pooling runs during
    conv0, and routing(1) is emitted near conv0's tail so conv1 starts
    with only a ~1us PE bubble.
  - PSUM drains alternate ACT (px0) / DVE (px1).
"""
import sys

if "/opt/trn_rl_repo" not in sys.path:
    sys.path.insert(0, "/opt/trn_rl_repo")

import ml_dtypes
import numpy as np

import concourse.bacc as bacc
import concourse.mybir as mybir
import concourse.tile as tile
from concourse.bass_utils import run_bass_kernel_spmd

F32 = mybir.dt.float32
BF16 = mybir.dt.bfloat16
AF = mybir.ActivationFunctionType
ALU = mybir.AluOpType
XY = mybir.AxisListType.XY

import os

Y_BF16 = os.environ.get("KV3_Y_BF16", "1") == "1"
X_BF16 = os.environ.get("KV3_X_BF16", "1") == "1"

N_CORES = 8
IMGS = 2
C_IN = 64
C_OUT = 64
H = 256
W = 256
E = 4
NTAP = 9
S = NTAP * C_OUT   # 576
RPT = 13           # rows per tile
NT = 10            # tiles per image (130 rows per half: -1..128 / 127..256)
HALF = 128
WP = 258           # padded row width
STAGE_ROWS = 16


def build_nc():
    nc = bacc.Bacc("TRN2", target_bir_lowering=False, debug=False,
                   num_devices=N_CORES)
    # xb: host-prepared bf16 tile layout, pads baked. Partition p<64:
    # top-half channels, row r = x row r-1 (row 0 = zero); p>=64:
    # bottom-half channels, row r = x row 127+r (row 129 = zero).
    # Col 0 and col 257 are zero.
    x = nc.dram_tensor("xb", [IMGS, 128, NT * RPT, WP],
                       BF16 if X_BF16 else F32, kind="ExternalInput")
    wt = nc.dram_tensor("wt", [128, E * S], BF16, kind="ExternalInput")
    fcw = nc.dram_tensor("fcw", [128, E], F32, kind="ExternalInput")
    fcb = nc.dram_tensor("fcb", [128, E], F32, kind="ExternalInput")
    y = nc.dram_tensor("y", [IMGS, C_OUT, H, W],
                       BF16 if Y_BF16 else F32, kind="ExternalOutput")

    with tile.TileContext(nc) as tc:
        with (
            tc.tile_pool(name="consts", bufs=1) as consts,
            tc.tile_pool(name="small", bufs=2) as small,
            tc.tile_pool(name="scratch", bufs=2) as scratch,
            tc.tile_pool(name="stage", bufs=2) as stage_pool,
            tc.tile_pool(name="psum", bufs=1, space="PSUM") as psum_pool,
        ):
            # ---- consts ----
            wtb = consts.tile([128, E * S], BF16)
            nc.sync.dma_start(wtb[:], wt[:])
            fcwt = consts.tile([128, E], F32)
            fcbt = consts.tile([128, E], F32)
            nc.sync.dma_start(fcwt[:], fcw[:])
            nc.sync.dma_start(fcbt[:], fcb[:])
            onest = consts.tile([128, 64], F32)
            nc.vector.memset(onest[:], 1.0)

            # prime the ACT table set (Sigmoid+Copy) during the fill
            sgz = consts.tile([128, 1], F32)
            sgo = consts.tile([128, 1], F32)
            nc.vector.memset(sgz[:], 0.0)
            nc.scalar.activation(sgo[:], sgz[:], AF.Sigmoid)

            # ---- persistent image tiles (loaded fully padded) ----
            xs = [[consts.tile([128, RPT, WP], BF16, name=f"xs{i}_{t}")
                   for t in range(NT)] for i in range(IMGS)]

            # per-image pooling partials: 11 cols used (tile0 -> 2)
            partials = [small.tile([128, 16], F32, name=f"par{i}", tag="par",
                                   bufs=2) for i in range(IMGS)]
            for i in range(IMGS):
                nc.vector.memset(partials[i][:], 0.0)

            def load_tile(i, t):
                if X_BF16:
                    nc.sync.dma_start(xs[i][t][:],
                                      x[i, :, RPT * t:RPT * (t + 1), :])
                else:
                    # SWDGE cast-on-load (bisection variant)
                    nc.gpsimd.dma_start(xs[i][t][:],
                                        x[i, :, RPT * t:RPT * (t + 1), :])

            def pool_tile(i, t):
                """Sum tile t of image i into partials[i]. Even tiles on
                DVE, odd on ACT. Tile 0: bottom rows 0,1 are x rows
                127,128, already counted by the top half - excluded."""
                par = partials[i]
                if t % 2 == 0:
                    if t == 0:
                        nc.vector.reduce_sum(par[:, 0:1], xs[i][0][:, 2:13, :],
                                             axis=XY)
                        nc.vector.reduce_sum(par[0:64, 1:2],
                                             xs[i][0][0:64, 0:2, :], axis=XY)
                    else:
                        nc.vector.reduce_sum(par[:, t + 1:t + 2],
                                             xs[i][t][:], axis=XY)
                else:
                    dum = scratch.tile([128, RPT, WP], BF16, tag="dum", bufs=2)
                    nc.scalar.activation(dum[:], xs[i][t][:], AF.Copy,
                                         accum_out=par[:, t + 1:t + 2])

            def routing_rt(i):
                """partials -> routing sigmoid tile rt [128, E]."""
                par = partials[i]
                pooled = small.tile([128, 1], F32, name="pooled")
                nc.vector.reduce_sum(pooled[:], par[:, 0:11],
                                     axis=mybir.AxisListType.X)
                tmp4 = small.tile([128, E], F32, name="tmp4")
                nc.vector.tensor_scalar(tmp4[:], fcwt[:], pooled[:, 0:1],
                                        1.0 / float(H * W),
                                        op0=ALU.mult, op1=ALU.mult)
                ps4 = psum_pool.tile([128, E], F32, name="ps4", tag="rt",
                                     bufs=1)
                # full sum (top+bottom) broadcast to both psum halves
                nc.tensor.matmul(ps4[0:64], onest[0:64, :], tmp4[0:64],
                                 start=True, stop=False, tile_position=(0, 0),
                                 skip_group_check=True)
                nc.tensor.matmul(ps4[0:64], onest[64:128, :], tmp4[64:128],
                                 start=False, stop=True, tile_position=(64, 0),
                                 skip_group_check=True)
                nc.tensor.matmul(ps4[64:128], onest[0:64, :], tmp4[0:64],
                                 start=True, stop=False, tile_position=(0, 64),
                                 skip_group_check=True)
                nc.tensor.matmul(ps4[64:128], onest[64:128, :], tmp4[64:128],
                                 start=False, stop=True,
                                 tile_position=(64, 64), skip_group_check=True)
                logits = small.tile([128, E], F32, name="logits")
                nc.vector.tensor_tensor(logits[:], ps4[:], fcbt[:], op=ALU.add)
                rt = small.tile([128, E], F32, name="rt", tag="rtt", bufs=2)
                nc.scalar.activation(rt[:], logits[:], AF.Sigmoid)
                return rt

            def new_wmix():
                return small.tile([128, S], BF16, name="wmix", tag="wmix",
                                  bufs=2)

            def wmix_step(rt, wmix, e):
                if e == 0:
                    nc.vector.tensor_scalar_mul(wmix[:], wtb[:, 0:S],
                                                rt[:, 0:1])
                else:
                    nc.vector.scalar_tensor_tensor(
                        wmix[:], wtb[:, e * S:(e + 1) * S], rt[:, e:e + 1],
                        wmix[:], op0=ALU.mult, op1=ALU.add)

            def conv(i, wmix, on_group=None):
                xi = xs[i]
                n_groups = 32           # 2 pairs per group
                gps = STAGE_ROWS // 4   # groups per stage tile (4)
                stage = None
                for g in range(n_groups):
                    if g % gps == 0:
                        stage = stage_pool.tile([128, STAGE_ROWS, W], BF16,
                                                name="stage", tag="st")
                    psA = psum_pool.tile([128, 2, W], F32, name="psA",
                                         tag="ps", bufs=6)
                    psB = psum_pool.tile([128, 2, W], F32, name="psB",
                                         tag="ps", bufs=6)
                    pstiles = (psA, psB)
                    # last tap must be unsplit for both pairs: pick clean kh
                    bad = set()
                    for px in range(2):
                        pair = 2 * g + px
                        for kh in range(3):
                            if (2 * pair + kh) % RPT == RPT - 1:
                                bad.add(kh)
                    clean = [kh for kh in range(3) if kh not in bad][-1]
                    khs = [kh for kh in range(3) if kh != clean] + [clean]
                    taps = [kh * 3 + kw for kh in khs for kw in range(3)]
                    for r, tap in enumerate(taps):
                        kh, kw = divmod(tap, 3)
                        st = r == 0
                        sp = r == len(taps) - 1
                        for px in range(2):
                            pair = 2 * g + px
                            L = 2 * pair + kh
                            t, m = divmod(L, RPT)
                            ps = pstiles[px]
                            for half in range(2):
                                hs = slice(0, 64) if half == 0 else \
                                    slice(64, 128)
                                lhsT = wmix[hs, tap * 64:(tap + 1) * 64]
                                if px == 0:
                                    tp = (0, 0) if half == 0 else (64, 64)
                                    osl = hs
                                else:
                                    tp = (0, 64) if half == 0 else (64, 0)
                                    osl = slice(64, 128) if half == 0 else \
                                        slice(0, 64)
                                if m <= RPT - 2:
                                    rhs = xi[t][hs, m:m + 2, kw:kw + 256]
                                    nc.tensor.matmul(
                                        ps[osl], lhsT, rhs, start=st, stop=sp,
                                        tile_position=tp,
                                        skip_group_check=True)
                                else:
                                    for j in range(2):
                                        tj, mj = divmod(L + j, RPT)
                                        rhs = xi[tj][hs, mj, kw:kw + 256]
                                        nc.tensor.matmul(
                                            ps[osl, j, :], lhsT, rhs,
                                            start=(st and j == 0), stop=sp,
                                            tile_position=tp,
                                            skip_group_check=True)
                    # drain psum -> staging: px0 on ACT, px1 on DVE
                    r0 = (g % gps) * 4
                    nc.scalar.activation(stage[:, r0:r0 + 2, :], psA[:],
                                         AF.Copy)
                    nc.vector.tensor_copy(stage[:, r0 + 2:r0 + 4, :], psB[:])
                    # stage full -> 4 interleaved store DMAs on gpsimd
                    if (g + 1) % gps == 0:
                        mrow = (g // gps) * STAGE_ROWS
                        nj = STAGE_ROWS // 4
                        sv = stage.rearrange("p (j b r) w -> p j b r w",
                                             j=nj, b=2, r=2)
                        ys = y[i].rearrange("c (blk four) w -> c blk four w",
                                            four=4)
                        nc.gpsimd.dma_start(
                            ys[:, mrow // 4:mrow // 4 + nj, 0:2, :],
                            sv[0:64, :, 0, :, :])
                        nc.gpsimd.dma_start(
                            ys[:, (HALF + mrow) // 4:(HALF + mrow) // 4 + nj,
                               2:4, :],
                            sv[0:64, :, 1, :, :])
                        nc.gpsimd.dma_start(
                            ys[:, (HALF + mrow) // 4:(HALF + mrow) // 4 + nj,
                               0:2, :],
                            sv[64:128, :, 0, :, :])
                        nc.gpsimd.dma_start(
                            ys[:, mrow // 4:mrow // 4 + nj, 2:4, :],
                            sv[64:128, :, 1, :, :])
                    if on_group is not None:
                        on_group(g)

            # ---- schedule ----
            for t in range(NT):
                load_tile(0, t)
                pool_tile(0, t)
            for t in range(NT):
                load_tile(1, t)
            rt0 = routing_rt(0)
            wmix0 = new_wmix()
            for e in range(E):
                wmix_step(rt0, wmix0, e)

            # image 1 pooling + routing interleaved into conv0's emission
            state = {"rt1": None, "wmix1": new_wmix()}

            def on_group(g):
                if 6 <= g <= 24 and g % 2 == 0:
                    pool_tile(1, (g - 6) // 2)
                elif g == 25:
                    pool_tile(1, 9)  # odd tile -> ACT, keeps DVE free
                elif g == 29:
                    state["rt1"] = routing_rt(1)
                elif g in (30, 31):
                    wmix_step(state["rt1"], state["wmix1"], g - 30)

            conv(0, wmix0, on_group=on_group)
            wmix_step(state["rt1"], state["wmix1"], 2)
            wmix_step(state["rt1"], state["wmix1"], 3)
            conv(1, state["wmix1"])

    nc.compile()
    return nc


_NC_CACHE = {}


def _get_nc():
    if "nc" not in _NC_CACHE:
        _NC_CACHE["nc"] = build_nc()
    return _NC_CACHE["nc"]


def _prep_x(x2b):
    """[2, 64, 256, 256] -> padded tile layout [2, 128, 130, 258]."""
    xp = np.zeros((IMGS, 128, NT * RPT, WP),
                  dtype=ml_dtypes.bfloat16 if X_BF16 else np.float32)
    xp[:, 0:64, 1:130, 1:257] = x2b[:, :, 0:129, :]
    xp[:, 64:128, 0:129, 1:257] = x2b[:, :, 127:256, :]
    return xp


def _prep_shared(weight, fc_w, fc_b):
    # [E, O, I, KH, KW] -> [I, E, KH, KW, O] -> [64, E*9*64], dup halves
    wt = np.ascontiguousarray(weight.transpose(2, 0, 3, 4, 1)).reshape(
        C_IN, E * NTAP * C_OUT)
    wt = np.concatenate([wt, wt], axis=0).astype(ml_dtypes.bfloat16)
    fcw = np.concatenate([fc_w.T, fc_w.T], axis=0).astype(np.float32)
    fcb = np.tile(fc_b.reshape(1, E), (128, 1)).astype(np.float32)
    return wt, fcw, fcb


def kernel(inputs, weight, fc_w, fc_b, stride=1, dilation=1, padding=1,
           _trace=False, _npx=2):
    assert int(stride) == 1 and int(dilation) == 1 and int(padding) == 1
    inputs = np.asarray(inputs, dtype=np.float32)
    B = inputs.shape[0]
    assert B == N_CORES * IMGS
    xb = inputs.astype(ml_dtypes.bfloat16) if X_BF16 else inputs
    wt, fcw, fcb = _prep_shared(np.asarray(weight), np.asarray(fc_w),
                                np.asarray(fc_b))
    nc = _get_nc()
    in_maps = []
    for c in range(N_CORES):
        in_maps.append({
            "xb": _prep_x(xb[2 * c:2 * c + 2]),
            "wt": wt, "fcw": fcw, "fcb": fcb,
        })
    res = run_bass_kernel_spmd(nc, in_maps, core_ids=list(range(N_CORES)),
                               trace=_trace)
    out = np.concatenate(
        [np.asarray(res.results[c]["y"]) for c in range(N_CORES)],
        axis=0).astype(np.float32)
    if _trace:
        return out, res
    return out


# revision 12
# speedup vs baseline: 1.1923x; 1.1550x over previous
"""CondConv2D Trainium2 kernel (v3).

Problem (hardcoded): B=16, C_in=64, H=W=256, E=4, C_out=64, 3x3, s=1, d=1, p=1.
Sharding: data-parallel over batch, 8 cores x 2 images.

v3 changes vs v2:
  - bf16 HBM I/O. Input is host-cast to bf16 in the padded tile layout
    (row AND col pads baked host-side), so loads are pure 128x6.7KB
    block copies: no device casts, no pad memsets, half the read
    traffic. Output dram tensor is bf16 (host upcasts to fp32): half
    the write traffic. Total HBM traffic 69MB -> 35MB.
  - Pooling is done by dedicated reduce ops split across DVE
    (tensor_reduce) and ACT (activation-Copy with accum_out into a
    scratch tile), interleaved with the loads / previous image's conv.
  - Routing fixed to sum top+bottom halves (4 ones-matmuls with
    cross tile_positions) instead of per-half logits.
  - Image 1 loads eagerly behind image 0 (bf16 halves the bandwidth
    demand, so loads+stores fit under conv0), its pooling runs during
    conv0, and routing(1) is emitted near conv0's tail so conv1 starts
    with only a ~1us PE bubble.
  - PSUM drains alternate ACT (px0) / DVE (px1).
"""
import sys

if "/opt/trn_rl_repo" not in sys.path:
    sys.path.insert(0, "/opt/trn_rl_repo")

import ml_dtypes
import numpy as np

import concourse.bacc as bacc
import concourse.mybir as mybir
import concourse.tile as tile
from concourse.bass_utils import run_bass_kernel_spmd

F32 = mybir.dt.float32
BF16 = mybir.dt.bfloat16
AF = mybir.ActivationFunctionType
ALU = mybir.AluOpType
XY = mybir.AxisListType.XY

import os

Y_BF16 = os.environ.get("KV3_Y_BF16", "1") == "1"
X_BF16 = os.environ.get("KV3_X_BF16", "1") == "1"

N_CORES = 8
IMGS = 2
C_IN = 64
C_OUT = 64
H = 256
W = 256
E = 4
NTAP = 9
S = NTAP * C_OUT   # 576
RPT = 13           # rows per tile
NT = 10            # tiles per image (130 rows per half: -1..128 / 127..256)
HALF = 128
WP = 258           # padded row width
STAGE_ROWS = 16


def build_nc():
    nc = bacc.Bacc("TRN2", target_bir_lowering=False, debug=False,
                   num_devices=N_CORES)
    # xb: host-prepared bf16 tile layout, pads baked. Partition p<64:
    # top-half channels, row r = x row r-1 (row 0 = zero); p>=64:
    # bottom-half channels, row r = x row 127+r (row 129 = zero).
    # Col 0 and col 257 are zero.
    x = nc.dram_tensor("xb", [IMGS, 128, NT * RPT, WP],
                       BF16 if X_BF16 else F32, kind="ExternalInput")
    wt = nc.dram_tensor("wt", [128, E * S], BF16, kind="ExternalInput")
    fcw = nc.dram_tensor("fcw", [128, E], F32, kind="ExternalInput")
    fcb = nc.dram_tensor("fcb", [128, E], F32, kind="ExternalInput")
    # Device-layout output: partition-major, host unscrambles.
    # Row R' = 4g+2b+j of partition p: see _unscramble_y.
    y = nc.dram_tensor("y", [IMGS, 128, H // 2, W],
                       BF16 if Y_BF16 else F32, kind="ExternalOutput")

    with tile.TileContext(nc) as tc:
        with (
            tc.tile_pool(name="consts", bufs=1) as consts,
            tc.tile_pool(name="small", bufs=2) as small,
            tc.tile_pool(name="scratch", bufs=2) as scratch,
            tc.tile_pool(name="stage", bufs=2) as stage_pool,
            tc.tile_pool(name="psum", bufs=1, space="PSUM") as psum_pool,
        ):
            # ---- consts ----
            wtb = consts.tile([128, E * S], BF16)
            nc.sync.dma_start(wtb[:], wt[:])
            fcwt = consts.tile([128, E], F32)
            fcbt = consts.tile([128, E], F32)
            nc.sync.dma_start(fcwt[:], fcw[:])
            nc.sync.dma_start(fcbt[:], fcb[:])
            onest = consts.tile([128, 64], F32)
            nc.vector.memset(onest[:], 1.0)

            # prime the ACT table set (Sigmoid+Copy) during the fill
            sgz = consts.tile([128, 1], F32)
            sgo = consts.tile([128, 1], F32)
            nc.vector.memset(sgz[:], 0.0)
            nc.scalar.activation(sgo[:], sgz[:], AF.Sigmoid)

            # ---- persistent image tiles (loaded fully padded) ----
            xs = [[consts.tile([128, RPT, WP], BF16, name=f"xs{i}_{t}")
                   for t in range(NT)] for i in range(IMGS)]

            # per-image pooling partials: 11 cols used (tile0 -> 2)
            partials = [small.tile([128, 16], F32, name=f"par{i}", tag="par",
                                   bufs=2) for i in range(IMGS)]
            for i in range(IMGS):
                nc.vector.memset(partials[i][:], 0.0)

            def load_tile(i, t):
                if X_BF16:
                    nc.sync.dma_start(xs[i][t][:],
                                      x[i, :, RPT * t:RPT * (t + 1), :])
                else:
                    # SWDGE cast-on-load (bisection variant)
                    nc.gpsimd.dma_start(xs[i][t][:],
                                        x[i, :, RPT * t:RPT * (t + 1), :])

            def pool_tile(i, t):
                """Sum tile t of image i into partials[i]. Even tiles on
                DVE, odd on ACT. Tile 0: bottom rows 0,1 are x rows
                127,128, already counted by the top half - excluded."""
                par = partials[i]
                if t % 2 == 0:
                    if t == 0:
                        nc.vector.reduce_sum(par[:, 0:1], xs[i][0][:, 2:13, :],
                                             axis=XY)
                        nc.vector.reduce_sum(par[0:64, 1:2],
                                             xs[i][0][0:64, 0:2, :], axis=XY)
                    else:
                        nc.vector.reduce_sum(par[:, t + 1:t + 2],
                                             xs[i][t][:], axis=XY)
                else:
                    dum = scratch.tile([128, RPT, WP], BF16, tag="dum", bufs=2)
                    nc.scalar.activation(dum[:], xs[i][t][:], AF.Copy,
                                         accum_out=par[:, t + 1:t + 2])

            def routing_rt(i):
                """partials -> routing sigmoid tile rt [128, E]."""
                par = partials[i]
                pooled = small.tile([128, 1], F32, name="pooled")
                nc.vector.reduce_sum(pooled[:], par[:, 0:11],
                                     axis=mybir.AxisListType.X)
                tmp4 = small.tile([128, E], F32, name="tmp4")
                nc.vector.tensor_scalar(tmp4[:], fcwt[:], pooled[:, 0:1],
                                        1.0 / float(H * W),
                                        op0=ALU.mult, op1=ALU.mult)
                ps4 = psum_pool.tile([128, E], F32, name="ps4", tag="rt",
                                     bufs=1)
                # full sum (top+bottom) broadcast to both psum halves
                nc.tensor.matmul(ps4[0:64], onest[0:64, :], tmp4[0:64],
                                 start=True, stop=False, tile_position=(0, 0),
                                 skip_group_check=True)
                nc.tensor.matmul(ps4[0:64], onest[64:128, :], tmp4[64:128],
                                 start=False, stop=True, tile_position=(64, 0),
                                 skip_group_check=True)
                nc.tensor.matmul(ps4[64:128], onest[0:64, :], tmp4[0:64],
                                 start=True, stop=False, tile_position=(0, 64),
                                 skip_group_check=True)
                nc.tensor.matmul(ps4[64:128], onest[64:128, :], tmp4[64:128],
                                 start=False, stop=True,
                                 tile_position=(64, 64), skip_group_check=True)
                logits = small.tile([128, E], F32, name="logits")
                nc.vector.tensor_tensor(logits[:], ps4[:], fcbt[:], op=ALU.add)
                rt = small.tile([128, E], F32, name="rt", tag="rtt", bufs=2)
                nc.scalar.activation(rt[:], logits[:], AF.Sigmoid)
                return rt

            def new_wmix():
                return small.tile([128, S], BF16, name="wmix", tag="wmix",
                                  bufs=2)

            def wmix_step(rt, wmix, e):
                if e == 0:
                    nc.vector.tensor_scalar_mul(wmix[:], wtb[:, 0:S],
                                                rt[:, 0:1])
                else:
                    nc.vector.scalar_tensor_tensor(
                        wmix[:], wtb[:, e * S:(e + 1) * S], rt[:, e:e + 1],
                        wmix[:], op0=ALU.mult, op1=ALU.add)

            def conv(i, wmix, on_group=None):
                xi = xs[i]
                n_groups = 32           # 2 pairs per group
                gps = STAGE_ROWS // 4   # groups per stage tile (4)
                stage = None
                for g in range(n_groups):
                    if g % gps == 0:
                        stage = stage_pool.tile([128, STAGE_ROWS, W], BF16,
                                                name="stage", tag="st")
                    psA = psum_pool.tile([128, 2, W], F32, name="psA",
                                         tag="ps", bufs=6)
                    psB = psum_pool.tile([128, 2, W], F32, name="psB",
                                         tag="ps", bufs=6)
                    pstiles = (psA, psB)
                    # last tap must be unsplit for both pairs: pick clean kh
                    bad = set()
                    for px in range(2):
                        pair = 2 * g + px
                        for kh in range(3):
                            if (2 * pair + kh) % RPT == RPT - 1:
                                bad.add(kh)
                    clean = [kh for kh in range(3) if kh not in bad][-1]
                    khs = [kh for kh in range(3) if kh != clean] + [clean]
                    taps = [kh * 3 + kw for kh in khs for kw in range(3)]
                    for r, tap in enumerate(taps):
                        kh, kw = divmod(tap, 3)
                        st = r == 0
                        sp = r == len(taps) - 1
                        for px in range(2):
                            pair = 2 * g + px
                            L = 2 * pair + kh
                            t, m = divmod(L, RPT)
                            ps = pstiles[px]
                            for half in range(2):
                                hs = slice(0, 64) if half == 0 else \
                                    slice(64, 128)
                                lhsT = wmix[hs, tap * 64:(tap + 1) * 64]
                                if px == 0:
                                    tp = (0, 0) if half == 0 else (64, 64)
                                    osl = hs
                                else:
                                    tp = (0, 64) if half == 0 else (64, 0)
                                    osl = slice(64, 128) if half == 0 else \
                                        slice(0, 64)
                                if m <= RPT - 2:
                                    rhs = xi[t][hs, m:m + 2, kw:kw + 256]
                                    nc.tensor.matmul(
                                        ps[osl], lhsT, rhs, start=st, stop=sp,
                                        tile_position=tp,
                                        skip_group_check=True)
                                else:
                                    for j in range(2):
                                        tj, mj = divmod(L + j, RPT)
                                        rhs = xi[tj][hs, mj, kw:kw + 256]
                                        nc.tensor.matmul(
                                            ps[osl, j, :], lhsT, rhs,
                                            start=(st and j == 0), stop=sp,
                                            tile_position=tp,
                                            skip_group_check=True)
                    # drain psum -> staging: px0 on ACT, px1 on DVE
                    r0 = (g % gps) * 4
                    nc.scalar.activation(stage[:, r0:r0 + 2, :], psA[:],
                                         AF.Copy)
                    nc.vector.tensor_copy(stage[:, r0 + 2:r0 + 4, :], psB[:])
                    # stage full -> one contiguous store DMA on gpsimd
                    # (128 descriptors x 8KB; host unscrambles the layout)
                    if (g + 1) % gps == 0:
                        mrow = (g // gps) * STAGE_ROWS
                        nc.gpsimd.dma_start(
                            y[i, :, mrow:mrow + STAGE_ROWS, :], stage[:])
                    if on_group is not None:
                        on_group(g)

            # ---- schedule ----
            for t in range(NT):
                load_tile(0, t)
                pool_tile(0, t)
            for t in range(NT):
                load_tile(1, t)
            rt0 = routing_rt(0)
            wmix0 = new_wmix()
            for e in range(E):
                wmix_step(rt0, wmix0, e)

            # image 1 pooling + routing interleaved into conv0's emission
            state = {"rt1": None, "wmix1": new_wmix()}

            def on_group(g):
                if 6 <= g <= 24 and g % 2 == 0:
                    pool_tile(1, (g - 6) // 2)
                elif g == 25:
                    pool_tile(1, 9)  # odd tile -> ACT, keeps DVE free
                elif g == 29:
                    state["rt1"] = routing_rt(1)
                elif g in (30, 31):
                    wmix_step(state["rt1"], state["wmix1"], g - 30)

            conv(0, wmix0, on_group=on_group)
            wmix_step(state["rt1"], state["wmix1"], 2)
            wmix_step(state["rt1"], state["wmix1"], 3)
            conv(1, state["wmix1"])

    nc.compile()
    return nc


_NC_CACHE = {}


def _get_nc():
    if "nc" not in _NC_CACHE:
        _NC_CACHE["nc"] = build_nc()
    return _NC_CACHE["nc"]


def _prep_x(x2b):
    """[2, 64, 256, 256] -> padded tile layout [2, 128, 130, 258]."""
    xp = np.zeros((IMGS, 128, NT * RPT, WP),
                  dtype=ml_dtypes.bfloat16 if X_BF16 else np.float32)
    xp[:, 0:64, 1:130, 1:257] = x2b[:, :, 0:129, :]
    xp[:, 64:128, 0:129, 1:257] = x2b[:, :, 127:256, :]
    return xp


def _unscramble_y(ydev):
    """[n, 128, 128, 256] device layout -> [n, 64, 256, 256].

    Device row R' = 4g+2b+j (g conv group, b px, j row-in-pair);
    partition p = P*64+c. b=0: P=0 -> y[c, 4g+j], P=1 -> y[c, 128+4g+j].
    b=1 (px1 psum halves swapped): P=1 -> y[c, 4g+2+j], P=0 -> 128+...
    """
    n = ydev.shape[0]
    ydv = ydev.reshape(n, 2, 64, 32, 2, 2, 256)   # [n, P, c, rr, b, j, w]
    out = np.empty((n, 64, 256, 256), dtype=ydev.dtype)
    yv = out.reshape(n, 64, 2, 32, 4, 256)        # [n, c, H2, rr, cls, w]
    yv[:, :, 0, :, 0:2] = ydv[:, 0, :, :, 0, :]
    yv[:, :, 1, :, 0:2] = ydv[:, 1, :, :, 0, :]
    yv[:, :, 0, :, 2:4] = ydv[:, 1, :, :, 1, :]
    yv[:, :, 1, :, 2:4] = ydv[:, 0, :, :, 1, :]
    return out


def _prep_shared(weight, fc_w, fc_b):
    # [E, O, I, KH, KW] -> [I, E, KH, KW, O] -> [64, E*9*64], dup halves
    wt = np.ascontiguousarray(weight.transpose(2, 0, 3, 4, 1)).reshape(
        C_IN, E * NTAP * C_OUT)
    wt = np.concatenate([wt, wt], axis=0).astype(ml_dtypes.bfloat16)
    fcw = np.concatenate([fc_w.T, fc_w.T], axis=0).astype(np.float32)
    fcb = np.tile(fc_b.reshape(1, E), (128, 1)).astype(np.float32)
    return wt, fcw, fcb


def kernel(inputs, weight, fc_w, fc_b, stride=1, dilation=1, padding=1,
           _trace=False, _npx=2):
    assert int(stride) == 1 and int(dilation) == 1 and int(padding) == 1
    inputs = np.asarray(inputs, dtype=np.float32)
    B = inputs.shape[0]
    assert B == N_CORES * IMGS
    xb = inputs.astype(ml_dtypes.bfloat16) if X_BF16 else inputs
    wt, fcw, fcb = _prep_shared(np.asarray(weight), np.asarray(fc_w),
                                np.asarray(fc_b))
    nc = _get_nc()
    in_maps = []
    for c in range(N_CORES):
        in_maps.append({
            "xb": _prep_x(xb[2 * c:2 * c + 2]),
            "wt": wt, "fcw": fcw, "fcb": fcb,
        })
    res = run_bass_kernel_spmd(nc, in_maps, core_ids=list(range(N_CORES)),
                               trace=_trace)
    ydev = np.concatenate(
        [np.asarray(res.results[c]["y"]) for c in range(N_CORES)], axis=0)
    out = _unscramble_y(ydev).astype(np.float32)
    if _trace:
        return out, res
    return out
